# revision 1
# baseline (speedup 1.0000x reference)
"""Trainium2 Bass kernel for nn_CrossModalGatedBottleneckAttention.

Contract: kernel(**inputs) takes the FULL unsharded inputs (as produced by
the problem's setup_inputs) and returns the full [16, 768, 512] output.

Strategy: pure data parallelism over the batch dim B=16 across 8
NeuronCores (2 batches per core); weights replicated per core. The
per-core pipeline keeps activations feature-major so every contraction
runs without operand transposes; softmaxes are token-major (free-axis,
with ACT accum_out row sums); attention-weight transposes use the PE
transpose path; matmuls run as float32r (FP22, full PE rate).
"""
import sys as _sys
for _p in ("/opt/trn_rl_repo",):
    if _p not in _sys.path:
        _sys.path.insert(0, _p)

import numpy as np
import concourse.bass as bass
import concourse.mybir as mybir
import concourse.tile as tile
from concourse.bass_utils import run_bass_kernel_spmd
from concourse.masks import make_identity

# ---------------------------------------------------------------------------
# Workaround for walrus sync-wait encoding limits: several instruction
# encodings in this neuronxcc build (TPB_CTRL drain, S3_LW self-loading
# fp32/fp32r matmul, ...) reject more than one sem-wait per instruction
# ("Too many sync wait commands"). After Tile scheduling, move all but one
# wait of each instruction onto same-engine NoOps inserted just before it.
# An engine blocks on each wait in order, so semantics are preserved.
_wsplit_ctr = [0]


def _split_waits(nc, max_waits=1):
    n_split = 0
    for f in nc.m.functions:
        for blk in f.blocks:
            insts = blk.instructions
            new_list = []
            changed = False
            for inst in insts:
                si = inst.sync_info
                if si is not None and si.on_wait and len(si.on_wait) > max_waits:
                    waits = list(si.on_wait)
                    extra, keep = waits[:-max_waits], waits[-max_waits:]
                    for w in extra:
                        _wsplit_ctr[0] += 1
                        nop = mybir.InstNoOp(
                            name=f"I-wsplit-{_wsplit_ctr[0]}", ins=[], outs=[])
                        nop.engine = inst.engine
                        nop.sync_info = mybir.SyncInfo(on_wait=[w], on_update=[])
                        new_list.append(nop)
                        n_split += 1
                    inst.sync_info = mybir.SyncInfo(
                        on_wait=keep, on_update=list(si.on_update or []))
                    changed = True
                new_list.append(inst)
            if changed:
                insts.clear()
                insts.extend(new_list)
    return n_split


# ---------------------------------------------------------------------------
# Kernel builder

F32 = mybir.dt.float32
F32R = mybir.dt.float32r
BF16 = mybir.dt.bfloat16
F16 = mybir.dt.float16
AF = mybir.ActivationFunctionType
ALU = mybir.AluOpType

F = 512
N = 768
NB = 256
NHEADS = 8
HD = 64
KT = F // 128          # 4 k-tiles over feature dim
NT = N // 128          # 6 tiles over tokens
NBT = NB // 128        # 2 tiles over bottleneck tokens
SCALE = float(F) ** -0.5
MHA_SCALE = float(HD) ** -0.5
B_LOC = 2              # batches per core

AV_DTYPE = "f16"       # "f32" exact | "f16"/"bf16" fast, for the MHA AV matmul


def mm_acc(nc, psum_ap, pairs):
    n = len(pairs)
    for i, (l, r) in enumerate(pairs):
        nc.tensor.matmul(psum_ap, l, r, start=(i == 0), stop=(i == n - 1))


def build(nc: bass.Bass, repeat: int = 1):
    dram = {}

    def din(name, shape):
        dram[name] = nc.declare_dram_parameter(name, list(shape), F32,
                                               isOutput=False)
        return dram[name]

    for name, shape in [
            ("x1t", [B_LOC, F, N]), ("x2t", [B_LOC, F, N]), ("zbt", [F, NB]),
            ("wkv_i", [F, 2 * F]), ("wq_j", [F, F]), ("wqkv_b", [F, 3 * F]),
            ("w_f", [2 * F, F]), ("b_f", [F]), ("w_m", [F, F]), ("b_m", [F]),
            ("w_qkv", [F, 3 * F]), ("w_proj", [F, F]), ("b_projr", [1, F]),
            ("onesr", [1, 128])]:
        din(name, shape)
    out = nc.declare_dram_parameter("out", [B_LOC, N, F], F32, isOutput=True)

    with tile.TileContext(nc) as tc:
        if repeat == 1:
            _body(nc, tc, dram, out)
        else:
            # benchmark mode: repeat the computation in-kernel so a single
            # dispatch amortizes host/proxy overhead
            with tc.For_i(0, repeat, 1):
                _body(nc, tc, dram, out)
    return nc


def _wview(ap):
    # [Fin, Fout] dram -> [128, Fin//128, Fout] partition-tiled f32r view
    return ap[:, :].rearrange("(k p) o -> p k o", p=128).bitcast(F32R)


def _xview(ap):
    # [F, N] dram -> [128, KT, N]
    return ap.rearrange("(k p) n -> p k n", p=128).bitcast(F32R)


def _body(nc, tc, dram, out):
    import contextlib
    with contextlib.ExitStack() as ctx:
        consts = ctx.enter_context(tc.tile_pool(name="consts", bufs=1))
        wts = ctx.enter_context(tc.tile_pool(name="wts", bufs=1))
        acts = ctx.enter_context(tc.tile_pool(name="acts", bufs=1))
        smalls = ctx.enter_context(tc.tile_pool(name="smalls", bufs=1))
        pp_mm = ctx.enter_context(tc.tile_pool(name="pp_mm", bufs=1, space="PSUM"))
        pp_tr = ctx.enter_context(tc.tile_pool(name="pp_tr", bufs=1, space="PSUM"))
        pp_st = ctx.enter_context(tc.tile_pool(name="pp_st", bufs=1, space="PSUM"))
        pp_h = ctx.enter_context(tc.tile_pool(name="pp_h", bufs=1, space="PSUM"))
        _emit(nc, tc, dram, out, consts, wts, acts, smalls,
              pp_mm, pp_tr, pp_st, pp_h)


def _emit(nc, tc, dram, out, consts, wts, acts, smalls,
          pp_mm, pp_tr, pp_st, pp_h):
    # ---- constants -------------------------------------------------
    ident = consts.tile([128, 128], F32)
    make_identity(nc, ident)
    bf_c = consts.tile([128, KT], F32)
    nc.sync.dma_start(out=bf_c, in_=dram["b_f"][:].rearrange("(k p) -> p k", p=128))
    bm_c = consts.tile([128, KT], F32)
    nc.sync.dma_start(out=bm_c, in_=dram["b_m"][:].rearrange("(k p) -> p k", p=128))
    bproj_r = consts.tile([1, F], F32R)
    nc.sync.dma_start(out=bproj_r, in_=dram["b_projr"][:, :].bitcast(F32R))
    ones_r = consts.tile([1, 128], F32R)
    nc.sync.dma_start(out=ones_r, in_=dram["onesr"][:, :].bitcast(F32R))

    # ---- prologue: bottleneck projections (batch independent) ------
    zbt_s = smalls.tile([128, KT, NB], F32R, tag="ctx_tm")
    nc.sync.dma_start(out=zbt_s, in_=_xview(dram["zbt"][:, :]))
    wqkvb_s = wts.tile([128, KT, 3 * F], F32R, tag="w1536")
    for _c in range(3):
        nc.sync.dma_start(out=wqkvb_s[:, :, _c * F:(_c + 1) * F],
                          in_=_wview(dram["wqkv_b"])[:, :, _c * F:(_c + 1) * F])

    q_bT = consts.tile([128, KT, NB], F32R)
    k_bT = consts.tile([128, KT, NB], F32R)
    for dst, co in ((q_bT, 0), (k_bT, F)):
        for mt in range(KT):
            ps = pp_mm.tile([128, 512], F32, tag="mm", bufs=3)
            mm_acc(nc, ps[:, :NB],
                   [(wqkvb_s[:, k, co + mt * 128: co + (mt + 1) * 128],
                     zbt_s[:, k, :]) for k in range(KT)])
            nc.vector.tensor_scalar_mul(dst[:, mt, :], ps[:, :NB], 0.2)
    v_b = consts.tile([128, NBT, F], F32R)
    for mt in range(NBT):
        ps = pp_mm.tile([128, 512], F32, tag="mm", bufs=3)
        mm_acc(nc, ps[:, :],
               [(zbt_s[:, k, mt * 128:(mt + 1) * 128],
                 wqkvb_s[:, k, 2 * F:3 * F]) for k in range(KT)])
        nc.vector.tensor_scalar_mul(v_b[:, mt, :], ps[:, :], 0.2)

    # ---- per-batch pipeline ---------------------------------------
    wqkv_s = None
    for b in range(B_LOC):
        # P1: inputs + projections
        z_it = acts.tile([128, KT, N], F32R, tag="z_it")
        nc.sync.dma_start(out=z_it, in_=_xview(dram["x1t"][b]))
        wkvi_s = wts.tile([128, KT, 2 * F], F32R, tag="wkvi")
        nc.sync.dma_start(out=wkvi_s, in_=_wview(dram["wkv_i"]))
        z_jt = acts.tile([128, KT, N], F32R, tag="bigD")
        nc.sync.dma_start(out=z_jt, in_=_xview(dram["x2t"][b]))
        wqj_s = wts.tile([128, KT, F], F32R, tag="w512")
        nc.sync.dma_start(out=wqj_s, in_=_wview(dram["wq_j"]))

        k_iT = acts.tile([128, KT, N], F32R, tag="bigA")
        q_jT = acts.tile([128, KT, N], F32R, tag="bigB")
        for dst, wsrc, xsrc in ((k_iT, wkvi_s, z_it), (q_jT, wqj_s, z_jt)):
            for mt in range(KT):
                for nh in range(2):
                    ps = pp_mm.tile([128, 512], F32, tag="mm", bufs=3)
                    mm_acc(nc, ps[:, :384],
                           [(wsrc[:, k, mt * 128:(mt + 1) * 128],
                             xsrc[:, k, nh * 384:(nh + 1) * 384]) for k in range(KT)])
                    nc.vector.tensor_copy(dst[:, mt, nh * 384:(nh + 1) * 384],
                                          ps[:, :384])
        v_i = smalls.tile([128, NT, F], F32R, tag="vtile")
        for nt in range(NT):
            ps = pp_mm.tile([128, 512], F32, tag="mm", bufs=3)
            mm_acc(nc, ps[:, :],
                   [(z_it[:, k, nt * 128:(nt + 1) * 128],
                     wkvi_s[:, k, F:2 * F]) for k in range(KT)])
            nc.scalar.copy(v_i[:, nt, :], ps[:, :])

        # P2: a_ib attention (tm softmax over kv = N tokens)
        E_ib = smalls.tile([128, NBT, N], F32, tag="etileS")
        for mt in range(NBT):
            sacc = smalls.tile([128, 2], F32, tag="sacc", bufs=2)
            for nh in range(2):
                ps = pp_mm.tile([128, 512], F32, tag="mm", bufs=3)
                mm_acc(nc, ps[:, :384],
                       [(q_bT[:, k, mt * 128:(mt + 1) * 128],
                         k_iT[:, k, nh * 384:(nh + 1) * 384]) for k in range(KT)])
                nc.scalar.activation(out=E_ib[:, mt, nh * 384:(nh + 1) * 384],
                                     in_=ps[:, :384], func=AF.Exp, scale=SCALE,
                                     accum_out=sacc[:, nh:nh + 1])
            ssum = smalls.tile([128, 2], F32, tag="ssum", bufs=2)
            nc.vector.tensor_add(ssum[:, 0:1], sacc[:, 0:1], sacc[:, 1:2])
            nc.vector.reciprocal(ssum[:, 1:2], ssum[:, 0:1])
            nc.vector.tensor_scalar_mul(E_ib[:, mt, :], E_ib[:, mt, :],
                                        ssum[:, 1:2])
        A_ibT = smalls.tile([128, NT, NB], F32R, tag="atile")
        for nt in range(NT):
            for mt in range(NBT):
                pt = pp_tr.tile([128, 128], F32, tag="tr", bufs=2)
                nc.tensor.transpose(pt[:, :], E_ib[:, mt, nt * 128:(nt + 1) * 128],
                                    ident[:, :])
                nc.vector.tensor_copy(A_ibT[:, nt, mt * 128:(mt + 1) * 128],
                                      pt[:, :])
        ctx_tm = smalls.tile([128, NBT, F], F32R, tag="ctx_tm")
        for mt in range(NBT):
            ps = pp_mm.tile([128, 512], F32, tag="mm", bufs=3)
            mm_acc(nc, ps[:, :],
                   [(A_ibT[:, nt, mt * 128:(mt + 1) * 128], v_i[:, nt, :])
                    for nt in range(NT)])
            nc.vector.tensor_copy(ctx_tm[:, mt, :], ps[:, :])
        ctx_fm = smalls.tile([128, KT, NB], F32R, tag="ctx_fm")
        for ft in range(KT):
            ps = pp_mm.tile([128, 512], F32, tag="mm", bufs=3)
            mm_acc(nc, ps[:, :NB],
                   [(v_i[:, nt, ft * 128:(ft + 1) * 128], A_ibT[:, nt, :])
                    for nt in range(NT)])
            nc.vector.tensor_copy(ctx_fm[:, ft, :], ps[:, :NB])

        # P3: the two [N, NB] attentions (tm softmax over NB free axis)
        A_upT = _small_attention(nc, smalls, pp_mm, pp_tr, ident,
                                 lhs=z_it, rhs_fm=ctx_fm, tag_a="atile2")
        A_bjT = _small_attention(nc, smalls, pp_mm, pp_tr, ident,
                                 lhs=q_jT, rhs_fm=k_bT, tag_a="atile3")

        # P4: a_ij_T fm (0.5 already folded into A_upT/A_bjT)
        aijT = acts.tile([128, KT, N], F32R, tag="bigD")
        for ft in range(KT):
            for nh in range(2):
                ps = pp_mm.tile([128, 512], F32, tag="mm", bufs=3)
                pairs = [(ctx_tm[:, mt, ft * 128:(ft + 1) * 128],
                          A_upT[:, mt, nh * 384:(nh + 1) * 384]) for mt in range(NBT)]
                pairs += [(v_b[:, mt, ft * 128:(ft + 1) * 128],
                           A_bjT[:, mt, nh * 384:(nh + 1) * 384]) for mt in range(NBT)]
                mm_acc(nc, ps[:, :384], pairs)
                nc.vector.tensor_copy(aijT[:, ft, nh * 384:(nh + 1) * 384],
                                      ps[:, :384])

        # P5: gated fusion
        wf_s = wts.tile([128, 2 * KT, F], F32R, tag="wkvi")
        nc.sync.dma_start(out=wf_s, in_=_wview(dram["w_f"]))
        wm_s = wts.tile([128, KT, F], F32R, tag="w512")
        nc.sync.dma_start(out=wm_s, in_=_wview(dram["w_m"]))
        z_jt2 = acts.tile([128, KT, N], F32R, tag="bigA")
        nc.sync.dma_start(out=z_jt2, in_=_xview(dram["x2t"][b]))
        f_T = smalls.tile([128, KT, N], F32, tag="vtile")
        for ft in range(KT):
            for nh in range(2):
                ps = pp_mm.tile([128, 512], F32, tag="mm", bufs=3)
                pairs = [(wf_s[:, k, ft * 128:(ft + 1) * 128],
                          aijT[:, k, nh * 384:(nh + 1) * 384]) for k in range(KT)]
                pairs += [(wf_s[:, KT + k, ft * 128:(ft + 1) * 128],
                           z_jt2[:, k, nh * 384:(nh + 1) * 384]) for k in range(KT)]
                mm_acc(nc, ps[:, :384], pairs)
                nc.scalar.activation(out=f_T[:, ft, nh * 384:(nh + 1) * 384],
                                     in_=ps[:, :384], func=AF.Sigmoid,
                                     bias=bf_c[:, ft:ft + 1], scale=1.0)
        h_T = acts.tile([128, KT, N], F32R, tag="bigB")
        for ft in range(KT):
            for nh in range(2):
                ps = pp_mm.tile([128, 512], F32, tag="mm", bufs=3)
                mm_acc(nc, ps[:, :384],
                       [(wm_s[:, k, ft * 128:(ft + 1) * 128],
                         aijT[:, k, nh * 384:(nh + 1) * 384]) for k in range(KT)])
                utmp = smalls.tile([128, 384], F32, tag="utmp", bufs=2)
                nc.vector.scalar_tensor_tensor(
                    out=utmp[:, :], in0=ps[:, :384], scalar=bm_c[:, ft:ft + 1],
                    in1=f_T[:, ft, nh * 384:(nh + 1) * 384],
                    op0=ALU.add, op1=ALU.mult)
                nc.vector.tensor_add(utmp[:, :], utmp[:, :],
                                     z_it[:, ft, nh * 384:(nh + 1) * 384].bitcast(F32))
                nc.vector.tensor_scalar_max(h_T[:, ft, nh * 384:(nh + 1) * 384],
                                            utmp[:, :], 0.0)

        # P6: MHA
        if wqkv_s is None:
            # replaces wqkv_b in the same slot; loaded once for both batches
            wqkv_s = wts.tile([128, KT, 3 * F], F32R, tag="w1536")
            nc.sync.dma_start(out=wqkv_s, in_=_wview(dram["w_qkv"]))
        wproj_s = wts.tile([128, KT, F], F32R, tag="w512")
        nc.sync.dma_start(out=wproj_s, in_=_wview(dram["w_proj"]))
        Q_T = acts.tile([128, KT, N], F32R, tag="bigA")
        K_T = acts.tile([128, KT, N], F32R, tag="bigC")
        for dst, co in ((Q_T, 0), (K_T, F)):
            for mt in range(KT):
                for nh in range(2):
                    ps = pp_mm.tile([128, 512], F32, tag="mm", bufs=3)
                    mm_acc(nc, ps[:, :384],
                           [(wqkv_s[:, k, co + mt * 128:co + (mt + 1) * 128],
                             h_T[:, k, nh * 384:(nh + 1) * 384]) for k in range(KT)])
                    nc.vector.tensor_copy(dst[:, mt, nh * 384:(nh + 1) * 384],
                                          ps[:, :384])
        avdt = {"f32": F32, "bf16": BF16, "f16": F16}[AV_DTYPE]
        zrow = smalls.tile([1, 390], avdt, tag="zrow")
        nc.vector.memset(zrow[:, :], 0.0)
        ones_h = smalls.tile([1, 128], avdt, tag="ones_h")
        nc.vector.memset(ones_h[:, :], 1.0)
        V_plus = smalls.tile([128, NT, NHEADS * (HD + 1)], avdt, tag="vtile2")
        Vp_h = V_plus.rearrange("p n (h c) -> p n h c", c=HD + 1)
        nc.vector.memset(Vp_h[:, :, :, HD], 1.0)
        for nt in range(NT):
            ps = pp_mm.tile([128, 512], F32, tag="mm", bufs=3)
            mm_acc(nc, ps[:, :],
                   [(h_T[:, k, nt * 128:(nt + 1) * 128],
                     wqkv_s[:, k, 2 * F:3 * F]) for k in range(KT)])
            nc.scalar.copy(Vp_h[:, nt, :, 0:HD],
                           ps[:, :].rearrange("p (h c) -> p h c", c=HD))

        # heads: kv-outer; q split into 512+256 chunks so the exp runs as
        # two large ACT ops per (head, kv); all 6 q-tiles' unnormalized
        # H columns (64 V cols + 1 colsum) accumulate in two PSUM tiles
        H_tm = smalls.tile([128, NT, F], F32, tag="vtile")
        QCH = ((0, 4), (4, 2))  # q chunks: 512 + 256 columns
        CHOFF = (0, 260)        # chunk column offsets inside the hps tile
        ROT = [(pp_st, "st", 1), (pp_tr, "tr", 2), (pp_tr, "tr", 2),
               (pp_mm, "mm", 3), (pp_mm, "mm", 3), (pp_mm, "mm", 3)]
        for h in range(NHEADS):
            po = 64 * (h % 2)
            kt = h // 2
            Qh = Q_T[po:po + 64, kt, :]
            Kh = K_T[po:po + 64, kt, :]
            hp = pp_h.tile([128, 390], F32, tag="h", bufs=2, name="hp")
            # start=True clears has_written for the whole PSUM bank, so the
            # interleaved per-qt accumulation groups can't each own a start;
            # zero the bank once, then every AV matmul accumulates
            nc.tensor.matmul(hp[:, :], ones_h[0:1, :], zrow[0:1, :],
                             start=True, stop=False, skip_group_check=True)
            for kv in range(NT):
                for ci, (q0, nq) in enumerate(QCH):
                    sti = (2 * kv + ci) % 6
                    stpool, sttag, stbufs = ROT[sti]
                    st = stpool.tile([128, 512], F32, tag=sttag, bufs=stbufs,
                                     name=f"st{sti}")
                    nc.tensor.matmul(st[:, :128 * nq],
                                     Kh[:, kv * 128:(kv + 1) * 128],
                                     Qh[:, q0 * 128:(q0 + nq) * 128],
                                     start=True, stop=True)
                    et = smalls.tile([128, 512], avdt, tag="et", bufs=6,
                                     name=f"et{ci}")
                    nc.scalar.activation(out=et[:, :128 * nq],
                                         in_=st[:, :128 * nq],
                                         func=AF.Exp, scale=MHA_SCALE)
                    for qt in range(nq):
                        nc.tensor.matmul(
                            hp[:, CHOFF[ci] + qt * 65:CHOFF[ci] + (qt + 1) * 65],
                            et[:, qt * 128:(qt + 1) * 128],
                            Vp_h[:, kv, h, :],
                            start=False, stop=(kv == NT - 1),
                            skip_group_check=True)
            for ci, (q0, nq) in enumerate(QCH):
                for qt in range(nq):
                    nq_glob = q0 + qt
                    rcp = smalls.tile([128, 1], F32, tag="rcp", bufs=2)
                    nc.vector.reciprocal(
                        rcp[:, :], hp[:, CHOFF[ci] + qt * 65 + 64:
                                      CHOFF[ci] + qt * 65 + 65])
                    nc.vector.tensor_scalar_mul(
                        H_tm[:, nq_glob, h * 64:(h + 1) * 64],
                        hp[:, CHOFF[ci] + qt * 65:CHOFF[ci] + qt * 65 + 64],
                        rcp[:, :])
        H_T = acts.tile([128, KT, N], F32R, tag="bigB")
        for nt in range(NT):
            for ft in range(KT):
                pt = pp_tr.tile([128, 128], F32, tag="tr", bufs=2)
                nc.tensor.transpose(pt[:, :], H_tm[:, nt, ft * 128:(ft + 1) * 128],
                                    ident[:, :])
                nc.vector.tensor_copy(H_T[:, ft, nt * 128:(nt + 1) * 128],
                                      pt[:, :])
        for nt in range(NT):
            ps = pp_mm.tile([128, 512], F32, tag="mm", bufs=3)
            pairs = [(H_T[:, k, nt * 128:(nt + 1) * 128], wproj_s[:, k, :])
                     for k in range(KT)]
            pairs.append((ones_r[0:1, :], bproj_r[0:1, :]))
            mm_acc(nc, ps[:, :], pairs)
            osb = smalls.tile([128, F], F32, tag="osb", bufs=2)
            nc.vector.tensor_copy(osb[:, :], ps[:, :])
            nc.sync.dma_start(out=out[b, nt * 128:(nt + 1) * 128, :], in_=osb[:, :])


def _small_attention(nc, smalls, pp_mm, pp_tr, ident, lhs, rhs_fm, tag_a):
    """A = 0.5 * softmax(lhs.T @ rhs_fm * SCALE, axis=-1); returns A.T
    [NB, N] f32r. lhs [128, KT, N] f32r fm; rhs_fm [128, KT, NB] f32r."""
    E = smalls.tile([128, NT, NB], F32, tag="etileS")
    for nt in range(NT):
        ps = pp_mm.tile([128, 512], F32, tag="mm", bufs=3)
        mm_acc(nc, ps[:, :NB],
               [(lhs[:, k, nt * 128:(nt + 1) * 128], rhs_fm[:, k, :])
                for k in range(KT)])
        srs = smalls.tile([128, 2], F32, tag="srs", bufs=3)
        nc.scalar.activation(out=E[:, nt, :], in_=ps[:, :NB], func=AF.Exp,
                             scale=SCALE, accum_out=srs[:, 0:1])
        nc.vector.reciprocal(srs[:, 1:2], srs[:, 0:1])
        nc.vector.tensor_scalar(out=E[:, nt, :], in0=E[:, nt, :],
                                scalar1=srs[:, 1:2], scalar2=0.5,
                                op0=ALU.mult, op1=ALU.mult)
    A_T = smalls.tile([128, NBT, N], F32R, tag=tag_a)
    for nt in range(NT):
        for mt in range(NBT):
            pt = pp_tr.tile([128, 128], F32, tag="tr", bufs=2)
            nc.tensor.transpose(pt[:, :], E[:, nt, mt * 128:(mt + 1) * 128],
                                ident[:, :])
            nc.vector.tensor_copy(A_T[:, mt, nt * 128:(nt + 1) * 128],
                                  pt[:, :])
    return A_T

# ---------------------------------------------------------------------------
# Host-side wrapper
N_CORES = 8
_nc_cache = {}


def _get_nc(repeat=1):
    if repeat not in _nc_cache:
        nc = bass.Bass("TRN2", num_devices=N_CORES)
        build(nc, repeat=repeat)
        _split_waits(nc)
        _nc_cache[repeat] = nc
    return _nc_cache[repeat]


def _host_prep_shared(inputs):
    f32 = np.float32

    def c(a):
        return np.ascontiguousarray(np.asarray(a, f32))

    return {
        "zbt": c(np.asarray(inputs["z_b"]).T),
        "wkv_i": c(np.asarray(inputs["Wqkv_i"])[:, F:]),
        "wq_j": c(np.asarray(inputs["Wqkv_j"])[:, :F]),
        "wqkv_b": c(inputs["Wqkv_b"]),
        "w_f": c(inputs["W_f"]), "b_f": c(inputs["b_f"]),
        "w_m": c(inputs["W_m"]), "b_m": c(inputs["b_m"]),
        "w_qkv": c(inputs["W_QKV"]), "w_proj": c(inputs["W_proj"]),
        "b_projr": c(np.asarray(inputs["b_proj"]).reshape(1, F)),
        "onesr": np.ones((1, 128), f32),
    }


def make_in_maps(inputs):
    x1 = np.asarray(inputs["x_1"], np.float32)
    x2 = np.asarray(inputs["x_2"], np.float32)
    B = x1.shape[0]
    assert B == N_CORES * B_LOC, (B, N_CORES, B_LOC)
    shared = _host_prep_shared(inputs)
    in_maps = []
    for c in range(N_CORES):
        sl = slice(c * B_LOC, (c + 1) * B_LOC)
        m = dict(shared)
        m["x1t"] = np.ascontiguousarray(x1[sl].transpose(0, 2, 1))
        m["x2t"] = np.ascontiguousarray(x2[sl].transpose(0, 2, 1))
        in_maps.append(m)
    return in_maps


def kernel(**inputs) -> np.ndarray:
    nc = _get_nc(repeat=1)
    in_maps = make_in_maps(inputs)
    res = run_bass_kernel_spmd(nc, in_maps, list(range(N_CORES)))
    out = np.concatenate([np.asarray(r["out"]) for r in res.results], axis=0)
    return out.astype(np.float32)

